# revision 34
# baseline (speedup 1.0000x reference)
"""AbsoluteLearnedPE kernel: data-parallel over batch B, one fused
GEMM-pair per core, PE-roofline-bound, with term-1 in fp8 DoubleRow.

Per core: logits_b = q_b @ E^T + E @ (k_b+E)^T with E = embed[:2048].
Term 1 (q@E^T) runs in fp8e4m3 with perf_mode=DoubleRow (2 contraction
rows per PE cell -> 4 matmuls of K=256 instead of 8 of K=128, ~1.8x
term-1 throughput); term 2 stays bf16. Measured l2 error of the hybrid
is ~1.7e-2 vs the 2e-2 gate (fp8 on BOTH terms is ~2.9e-2 - fails).

Host pre-computes transposes, the k+E add, the bf16/fp8 casts, laid out
stripe-major [KS, 128, DTILES, 512] so a DMA descriptor is [128,
contiguous d-span]. Output is written bf16 and upconverted on host.

Schedule notes (exec window = first useful instruction -> last
instruction end; the ~9.7us end-of-NEFF semaphore-file restore is fixed
overhead):
- DMA queues deliver ~1 descriptor per ~2.2-2.5us nearly independent of
  size, so descriptors are the largest the consumption frontier allows:
  d-pairs for stripe 0, half/whole stripes later (47 input descriptors).
- memset-seeded PE warmups bridge from the prologue to first-chunk
  arrival (a PE idle gap resets the HAM clock ramp; re-warming costs
  ~2.5us at 4/8 rate).
- final wave runs its 4 groups sequentially so 3 of 4 evictions hide
  under matmuls; its output DMAs fan out across queues.
"""

import numpy as np

B, Q, K, D = 8, 2048, 2048, 1024
DTILES = D // 128     # 8
QT = Q // 128         # 16
KSTRIPE = 512
KS = K // KSTRIPE     # 4
WARM_MMS = 15

_CACHE = {}
TRACE = False


def _build():
    from concourse import bacc
    import concourse.mybir as mybir
    import concourse.tile as tile

    f32 = mybir.dt.float32
    bf16 = mybir.dt.bfloat16
    fp8 = mybir.dt.float8e4
    DR = mybir.MatmulPerfMode.DoubleRow

    nc = bacc.Bacc("TRN2", target_bir_lowering=False, debug=False, num_devices=B)
    q8Ts = nc.dram_tensor("q8Ts", [KS, 128, DTILES, KSTRIPE], fp8,
                          kind="ExternalInput").ap()
    e8Ts = nc.dram_tensor("e8Ts", [KS, 128, DTILES, KSTRIPE], fp8,
                          kind="ExternalInput").ap()
    eTs = nc.dram_tensor("eTs", [KS, 128, DTILES, KSTRIPE], bf16,
                         kind="ExternalInput").ap()
    kpeTs = nc.dram_tensor("kpeTs", [KS, 128, DTILES, KSTRIPE], bf16,
                           kind="ExternalInput").ap()
    out16 = nc.dram_tensor("out16", [Q, K], bf16, kind="ExternalOutput").ap()

    with tile.TileContext(nc) as tc:
        with tc.tile_pool(name="big", bufs=1) as big, \
             tc.tile_pool(name="outp", bufs=12) as outp, \
             tc.tile_pool(name="mps", bufs=8, space="PSUM") as mps:

            q8_sb = big.tile([128, KS, DTILES, KSTRIPE], fp8, tag="q8T")
            e8_sb = big.tile([128, KS, DTILES, KSTRIPE], fp8, tag="e8T")
            e_sb = big.tile([128, KS, DTILES, KSTRIPE], bf16, tag="eT")
            kpe_sb = big.tile([128, KS, DTILES, KSTRIPE], bf16, tag="kpeT")

            # PE warmups bridge the DMA lead-in, keeping the HAM clock
            # ramp alive until the first chunks land. (No separate
            # priming descriptors: block A's first pairs absorb the DGE
            # cold-start themselves, and every later descriptor then
            # lands one ~2.3us queue-cadence slot earlier.)
            wtile = big.tile([128, KSTRIPE], bf16, tag="warm")
            nc.gpsimd.memset(wtile[:], 0.0)
            wps = mps.tile([128, KSTRIPE], f32, tag="mps")
            for _ in range(WARM_MMS):
                nc.tensor.matmul(wps[:], wtile[:, 0:128], wtile[:],
                                 start=True, stop=True)

            rr = {"i": 0}
            engs = [nc.sync, nc.scalar, nc.gpsimd]

            def load(sb, dram, s, dlo, dhi):
                eng = engs[rr["i"] % 3]
                rr["i"] += 1
                eng.dma_start(out=sb[:, s, dlo:dhi, :], in_=dram[s, :, dlo:dhi, :])

            # block A: stripe 0 in consumption order. Each group step
            # runs t2 (bf16, needs e+kpe pairs) first and t1 (fp8-DR,
            # needs q8/e8 quads) ~3us later, so the bf16 pairs lead and
            # the half-size fp8 quads slot in behind them. 256KB pair
            # descriptors are the sweet spot: the first descriptor's
            # cold-start latency grows with size (512KB firsts land ~23us
            # and stall the stream), while finer splits lose to the
            # ~2.3us/descriptor queue cadence.
            load(e_sb, eTs, 0, 0, 2)
            load(kpe_sb, kpeTs, 0, 0, 2)
            load(q8_sb, q8Ts, 0, 0, 4)
            load(e8_sb, e8Ts, 0, 0, 4)
            load(e_sb, eTs, 0, 2, 4)
            load(kpe_sb, kpeTs, 0, 2, 4)
            load(e_sb, eTs, 0, 4, 6)
            load(kpe_sb, kpeTs, 0, 4, 6)
            load(q8_sb, q8Ts, 0, 4, 8)
            load(e8_sb, e8Ts, 0, 4, 8)
            load(e_sb, eTs, 0, 6, 8)
            load(kpe_sb, kpeTs, 0, 6, 8)
            # block B: wave (0,qt4-7) lhsT needs q8 s1 (t1) + e s1 (t2).
            # q8 s1 is split in two halves on different queues so the
            # wave's first t1 d-pairs gate on the earlier half only —
            # halves the jitter exposure against its ~25us deadline.
            load(q8_sb, q8Ts, 1, 0, 4)
            load(q8_sb, q8Ts, 1, 4, 8)
            load(e_sb, eTs, 1, 0, 4)
            load(e_sb, eTs, 1, 4, 8)
            # blocks C-E: whole-stripe DMAs, need-order.
            load(e8_sb, e8Ts, 1, 0, 8)       # wave (1,0) t1 rhs
            load(kpe_sb, kpeTs, 1, 0, 8)     # wave (1,0) t2 rhs
            load(q8_sb, q8Ts, 2, 0, 8)       # wave (0,8) lhsT
            load(e_sb, eTs, 2, 0, 8)
            load(q8_sb, q8Ts, 3, 0, 8)       # wave (0,12) lhsT
            load(e_sb, eTs, 3, 0, 8)
            load(e8_sb, e8Ts, 2, 0, 8)       # waves (2,*) rhs
            load(kpe_sb, kpeTs, 2, 0, 8)
            load(e8_sb, e8Ts, 3, 0, 8)       # waves (3,*) rhs
            load(kpe_sb, kpeTs, 3, 0, 8)

            def emit_out(pso, ks, qt, j=0, late=False, dma_eng=None):
                o_t = outp.tile([128, KSTRIPE], bf16, tag="o_t", name="o_t")
                if late and j % 2 == 1:
                    nc.scalar.copy(out=o_t[:], in_=pso[:])
                else:
                    nc.vector.tensor_copy(out=o_t[:], in_=pso[:])
                (dma_eng or nc.sync).dma_start(
                    out=out16[qt * 128:(qt + 1) * 128,
                              ks * KSTRIPE:(ks + 1) * KSTRIPE],
                    in_=o_t[:])

            def t2_mm(pso, ks, qt, d, start=False, stop=False):
                sq, cq = qt // 4, qt % 4
                qs = slice(cq * 128, (cq + 1) * 128)
                nc.tensor.matmul(pso[:], e_sb[:, sq, d, qs],
                                 kpe_sb[:, ks, d, :], start=start, stop=stop)

            def t1_mm(pso, ks, qt, dp, start=False, stop=False):
                sq, cq = qt // 4, qt % 4
                qs = slice(cq * 128, (cq + 1) * 128)
                d = 2 * dp
                nc.tensor.matmul(pso[:], q8_sb[:, sq, d:d + 2, qs],
                                 e8_sb[:, ks, d:d + 2, :],
                                 start=start, stop=stop, perf_mode=DR)

            def group_mms(pso, ks, qt, dp):
                # One d-pair step of one group: 2 bf16 matmuls (t2) then
                # 1 fp8-DR matmul (t1, contraction 256). t2 leads because
                # its chunks arrive first during the ramp.
                d = 2 * dp
                t2_mm(pso, ks, qt, d, start=(dp == 0))
                t2_mm(pso, ks, qt, d + 1)
                t1_mm(pso, ks, qt, dp, stop=(dp == 3))

            def wave(ks, qt_base, late=False, t1_first=False):
                # 4 groups, d-pair-major interleaved across groups so each
                # delivered chunk set unlocks 12 matmuls: the 8 t2 mms of
                # a step run before its 4 t1 mms. t1_first flips that —
                # used for wave (0,4), whose t1 needs only the small q8 s1
                # block (rhs e8 s0 is resident) while its t2 lhsT (e s1,
                # 1MB halves) lands ~3us later.
                qts = [qt_base + j for j in range(4)]
                psos = [mps.tile([128, KSTRIPE], f32, tag="mps",
                                 name=f"pso_{ks}_{qt}") for qt in qts]
                if t1_first:
                    for dp in range(4):
                        for j, qt in enumerate(qts):
                            t1_mm(psos[j], ks, qt, dp, start=(dp == 0))
                    for d in range(DTILES):
                        for j, qt in enumerate(qts):
                            t2_mm(psos[j], ks, qt, d, stop=(d == DTILES - 1))
                else:
                    for dp in range(4):
                        d = 2 * dp
                        for j, qt in enumerate(qts):
                            t2_mm(psos[j], ks, qt, d, start=(dp == 0))
                            t2_mm(psos[j], ks, qt, d + 1)
                        for j, qt in enumerate(qts):
                            t1_mm(psos[j], ks, qt, dp, stop=(dp == 3))
                for j, qt in enumerate(qts):
                    emit_out(psos[j], ks, qt, j, late)

            def emit_out_split(pso, ks, qt):
                # Minimal-latency eviction for the very last group: two
                # half-tiles cast on Vector+Scalar in parallel, two
                # half-DMAs on Sync+GpSimd in parallel — halves the
                # post-stream drain vs the single-tile path.
                h = KSTRIPE // 2
                o_t = outp.tile([128, KSTRIPE], bf16, tag="o_t", name="o_t")
                nc.vector.tensor_copy(out=o_t[:, 0:h], in_=pso[:, 0:h])
                nc.scalar.copy(out=o_t[:, h:], in_=pso[:, h:])
                rows = slice(qt * 128, (qt + 1) * 128)
                nc.sync.dma_start(
                    out=out16[rows, ks * KSTRIPE:ks * KSTRIPE + h],
                    in_=o_t[:, 0:h])
                nc.gpsimd.dma_start(
                    out=out16[rows, ks * KSTRIPE + h:(ks + 1) * KSTRIPE],
                    in_=o_t[:, h:])

            def wave_seq(ks, qt_base):
                # Final wave: groups sequential so evictions overlap the
                # stream; output DMAs fan out across queues.
                dma_engs = [nc.sync, nc.gpsimd, nc.gpsimd, nc.sync]
                for j, qt in enumerate(qt_base + jj for jj in range(4)):
                    pso = mps.tile([128, KSTRIPE], f32, tag="mps",
                                   name=f"pso_{ks}_{qt}")
                    for dp in range(4):
                        group_mms(pso, ks, qt, dp)
                    if j == 3:
                        emit_out_split(pso, ks, qt)
                    else:
                        emit_out(pso, ks, qt, j, late=True, dma_eng=dma_engs[j])

            WAVES = [(0, 0), (0, 4), (1, 0), (1, 4),
                     (0, 8), (1, 8), (0, 12), (1, 12),
                     (2, 0), (2, 4), (2, 8), (2, 12),
                     (3, 0), (3, 4), (3, 8)]
            for wi, (ks, qt_base) in enumerate(WAVES):
                wave(ks, qt_base, late=(wi >= 4), t1_first=(wi == 1))
            wave_seq(3, 12)
    nc.compile()
    return nc


def _stripe_major(x16: np.ndarray) -> np.ndarray:
    # [D, K] -> [KS, 128, DTILES, 512] with [s, p, d, c] = x[d*128+p, s*512+c]
    return np.ascontiguousarray(
        x16.reshape(DTILES, 128, KS, KSTRIPE).transpose(2, 1, 0, 3))


def kernel(q: np.ndarray, k: np.ndarray, embed: np.ndarray) -> np.ndarray:
    import ml_dtypes
    from concourse.bass_utils import run_bass_kernel_spmd

    if "nc" not in _CACHE:
        _CACHE["nc"] = _build()
    nc = _CACHE["nc"]

    bf = ml_dtypes.bfloat16
    f8 = ml_dtypes.float8_e4m3fn
    e = np.asarray(embed[:K], dtype=np.float32)
    eT = e.T
    e8Ts = _stripe_major(eT.astype(f8))
    eTs = _stripe_major(eT.astype(bf))
    in_maps = []
    for b in range(B):
        q8Ts = _stripe_major(np.asarray(q[b], dtype=np.float32).T.astype(f8))
        kpeTs = _stripe_major((np.asarray(k[b], dtype=np.float32) + e).T.astype(bf))
        in_maps.append({"q8Ts": q8Ts, "e8Ts": e8Ts, "eTs": eTs, "kpeTs": kpeTs})
    res = run_bass_kernel_spmd(nc, in_maps, core_ids=list(range(B)), trace=TRACE)
    _CACHE["last_result"] = res
    return np.stack([res.results[b]["out16"].astype(np.float32) for b in range(B)])


# revision 35
# speedup vs baseline: 1.1961x; 1.1961x over previous
"""AbsoluteLearnedPE kernel: data-parallel over batch B, one fused
GEMM-pair per core, PE-roofline-bound, with term-1 in fp8 DoubleRow.

Per core: logits_b = q_b @ E^T + E @ (k_b+E)^T with E = embed[:2048].
Term 1 (q@E^T) runs in fp8e4m3 with perf_mode=DoubleRow (2 contraction
rows per PE cell -> 4 matmuls of K=256 instead of 8 of K=128, ~1.8x
term-1 throughput); term 2 stays bf16. Measured l2 error of the hybrid
is ~1.7e-2 vs the 2e-2 gate (fp8 on BOTH terms is ~2.9e-2 - fails).

Host pre-computes transposes, the k+E add, the bf16/fp8 casts, laid out
stripe-major [KS, 128, DTILES, 512] so a DMA descriptor is [128,
contiguous d-span]. Output is written bf16 and upconverted on host.

Schedule notes (exec window = first useful instruction -> last
instruction end; the ~9.7us end-of-NEFF semaphore-file restore is fixed
overhead):
- DMA queues deliver ~1 descriptor per ~2.2-2.5us nearly independent of
  size, so descriptors are the largest the consumption frontier allows:
  d-pairs for stripe 0, half/whole stripes later (47 input descriptors).
- memset-seeded PE warmups bridge from the prologue to first-chunk
  arrival (a PE idle gap resets the HAM clock ramp; re-warming costs
  ~2.5us at 4/8 rate).
- final wave runs its 4 groups sequentially so 3 of 4 evictions hide
  under matmuls; its output DMAs fan out across queues.
"""

import numpy as np

B, Q, K, D = 8, 2048, 2048, 1024
DTILES = D // 128     # 8
QT = Q // 128         # 16
KSTRIPE = 512
KS = K // KSTRIPE     # 4
WARM_MMS = 15

_CACHE = {}
TRACE = False


def _build():
    from concourse import bacc
    import concourse.mybir as mybir
    import concourse.tile as tile

    f32 = mybir.dt.float32
    bf16 = mybir.dt.bfloat16
    fp8 = mybir.dt.float8e4
    DR = mybir.MatmulPerfMode.DoubleRow

    nc = bacc.Bacc("TRN2", target_bir_lowering=False, debug=False, num_devices=B)
    q8Ts = nc.dram_tensor("q8Ts", [KS, 128, DTILES, KSTRIPE], fp8,
                          kind="ExternalInput").ap()
    e8Ts = nc.dram_tensor("e8Ts", [KS, 128, DTILES, KSTRIPE], fp8,
                          kind="ExternalInput").ap()
    eTs = nc.dram_tensor("eTs", [KS, 128, DTILES, KSTRIPE], bf16,
                         kind="ExternalInput").ap()
    kpeTs = nc.dram_tensor("kpeTs", [KS, 128, DTILES, KSTRIPE], bf16,
                           kind="ExternalInput").ap()
    out16 = nc.dram_tensor("out16", [Q, K], bf16, kind="ExternalOutput").ap()

    with tile.TileContext(nc) as tc:
        with tc.tile_pool(name="big", bufs=1) as big, \
             tc.tile_pool(name="outp", bufs=12) as outp, \
             tc.tile_pool(name="mps", bufs=8, space="PSUM") as mps:

            q8_sb = big.tile([128, KS, DTILES, KSTRIPE], fp8, tag="q8T")
            e8_sb = big.tile([128, KS, DTILES, KSTRIPE], fp8, tag="e8T")
            e_sb = big.tile([128, KS, DTILES, KSTRIPE], bf16, tag="eT")
            kpe_sb = big.tile([128, KS, DTILES, KSTRIPE], bf16, tag="kpeT")

            # PE warmups bridge the DMA lead-in, keeping the HAM clock
            # ramp alive until the first chunks land. (No separate
            # priming descriptors: block A's first pairs absorb the DGE
            # cold-start themselves, and every later descriptor then
            # lands one ~2.3us queue-cadence slot earlier.)
            wtile = big.tile([128, KSTRIPE], bf16, tag="warm")
            nc.gpsimd.memset(wtile[:], 0.0)
            wps = mps.tile([128, KSTRIPE], f32, tag="mps")
            for _ in range(WARM_MMS):
                nc.tensor.matmul(wps[:], wtile[:, 0:128], wtile[:],
                                 start=True, stop=True)

            rr = {"i": 0}
            engs = [nc.sync, nc.scalar, nc.gpsimd]

            def load(sb, dram, s, dlo, dhi):
                eng = engs[rr["i"] % 3]
                rr["i"] += 1
                eng.dma_start(out=sb[:, s, dlo:dhi, :], in_=dram[s, :, dlo:dhi, :])

            # block A: stripe 0 in consumption order. Each group step
            # runs t2 (bf16, needs e+kpe pairs) first and t1 (fp8-DR,
            # needs q8/e8 quads) ~3us later, so the bf16 pairs lead and
            # the half-size fp8 quads slot in behind them. 256KB pair
            # descriptors are the sweet spot: the first descriptor's
            # cold-start latency grows with size (512KB firsts land ~23us
            # and stall the stream), while finer splits lose to the
            # ~2.3us/descriptor queue cadence.
            load(e_sb, eTs, 0, 0, 2)
            load(kpe_sb, kpeTs, 0, 0, 2)
            load(q8_sb, q8Ts, 0, 0, 4)
            load(e8_sb, e8Ts, 0, 0, 4)
            load(e_sb, eTs, 0, 2, 4)
            load(kpe_sb, kpeTs, 0, 2, 4)
            load(e_sb, eTs, 0, 4, 6)
            load(kpe_sb, kpeTs, 0, 4, 6)
            load(q8_sb, q8Ts, 0, 4, 8)
            load(e8_sb, e8Ts, 0, 4, 8)
            load(e_sb, eTs, 0, 6, 8)
            load(kpe_sb, kpeTs, 0, 6, 8)
            # block B: wave (0,qt4-7) lhsT needs q8 s1 (t1) + e s1 (t2).
            load(q8_sb, q8Ts, 1, 0, 8)
            load(e_sb, eTs, 1, 0, 4)
            load(e_sb, eTs, 1, 4, 8)
            # blocks C-E: whole-stripe DMAs, need-order.
            load(e8_sb, e8Ts, 1, 0, 8)       # wave (1,0) t1 rhs
            load(kpe_sb, kpeTs, 1, 0, 8)     # wave (1,0) t2 rhs
            load(q8_sb, q8Ts, 2, 0, 8)       # wave (0,8) lhsT
            load(e_sb, eTs, 2, 0, 8)
            load(q8_sb, q8Ts, 3, 0, 8)       # wave (0,12) lhsT
            load(e_sb, eTs, 3, 0, 8)
            load(e8_sb, e8Ts, 2, 0, 8)       # waves (2,*) rhs
            load(kpe_sb, kpeTs, 2, 0, 8)
            load(e8_sb, e8Ts, 3, 0, 8)       # waves (3,*) rhs
            load(kpe_sb, kpeTs, 3, 0, 8)

            def emit_out(pso, ks, qt, j=0, late=False, dma_eng=None):
                o_t = outp.tile([128, KSTRIPE], bf16, tag="o_t", name="o_t")
                if late and j % 2 == 1:
                    nc.scalar.copy(out=o_t[:], in_=pso[:])
                else:
                    nc.vector.tensor_copy(out=o_t[:], in_=pso[:])
                (dma_eng or nc.sync).dma_start(
                    out=out16[qt * 128:(qt + 1) * 128,
                              ks * KSTRIPE:(ks + 1) * KSTRIPE],
                    in_=o_t[:])

            def t2_mm(pso, ks, qt, d, start=False, stop=False):
                sq, cq = qt // 4, qt % 4
                qs = slice(cq * 128, (cq + 1) * 128)
                nc.tensor.matmul(pso[:], e_sb[:, sq, d, qs],
                                 kpe_sb[:, ks, d, :], start=start, stop=stop)

            def t1_mm(pso, ks, qt, dp, start=False, stop=False):
                sq, cq = qt // 4, qt % 4
                qs = slice(cq * 128, (cq + 1) * 128)
                d = 2 * dp
                nc.tensor.matmul(pso[:], q8_sb[:, sq, d:d + 2, qs],
                                 e8_sb[:, ks, d:d + 2, :],
                                 start=start, stop=stop, perf_mode=DR)

            def group_mms(pso, ks, qt, dp):
                # One d-pair step of one group: 2 bf16 matmuls (t2) then
                # 1 fp8-DR matmul (t1, contraction 256). t2 leads because
                # its chunks arrive first during the ramp.
                d = 2 * dp
                t2_mm(pso, ks, qt, d, start=(dp == 0))
                t2_mm(pso, ks, qt, d + 1)
                t1_mm(pso, ks, qt, dp, stop=(dp == 3))

            def wave(ks, qt_base, late=False, t1_first=False):
                # 4 groups, d-pair-major interleaved across groups so each
                # delivered chunk set unlocks 12 matmuls: the 8 t2 mms of
                # a step run before its 4 t1 mms. t1_first flips that —
                # used for wave (0,4), whose t1 needs only the small q8 s1
                # block (rhs e8 s0 is resident) while its t2 lhsT (e s1,
                # 1MB halves) lands ~3us later.
                qts = [qt_base + j for j in range(4)]
                psos = [mps.tile([128, KSTRIPE], f32, tag="mps",
                                 name=f"pso_{ks}_{qt}") for qt in qts]
                if t1_first:
                    for dp in range(4):
                        for j, qt in enumerate(qts):
                            t1_mm(psos[j], ks, qt, dp, start=(dp == 0))
                    for d in range(DTILES):
                        for j, qt in enumerate(qts):
                            t2_mm(psos[j], ks, qt, d, stop=(d == DTILES - 1))
                else:
                    for dp in range(4):
                        d = 2 * dp
                        for j, qt in enumerate(qts):
                            t2_mm(psos[j], ks, qt, d, start=(dp == 0))
                            t2_mm(psos[j], ks, qt, d + 1)
                        for j, qt in enumerate(qts):
                            t1_mm(psos[j], ks, qt, dp, stop=(dp == 3))
                for j, qt in enumerate(qts):
                    emit_out(psos[j], ks, qt, j, late)

            def emit_out_split(pso, ks, qt):
                # Minimal-latency eviction for the very last group: two
                # half-tiles cast on Vector+Scalar in parallel, two
                # half-DMAs on Sync+GpSimd in parallel — halves the
                # post-stream drain vs the single-tile path.
                h = KSTRIPE // 2
                o_t = outp.tile([128, KSTRIPE], bf16, tag="o_t", name="o_t")
                nc.vector.tensor_copy(out=o_t[:, 0:h], in_=pso[:, 0:h])
                nc.scalar.copy(out=o_t[:, h:], in_=pso[:, h:])
                rows = slice(qt * 128, (qt + 1) * 128)
                nc.sync.dma_start(
                    out=out16[rows, ks * KSTRIPE:ks * KSTRIPE + h],
                    in_=o_t[:, 0:h])
                nc.gpsimd.dma_start(
                    out=out16[rows, ks * KSTRIPE + h:(ks + 1) * KSTRIPE],
                    in_=o_t[:, h:])

            def wave_seq(ks, qt_base):
                # Final wave: groups sequential so evictions overlap the
                # stream; output DMAs fan out across queues.
                dma_engs = [nc.sync, nc.gpsimd, nc.gpsimd, nc.sync]
                for j, qt in enumerate(qt_base + jj for jj in range(4)):
                    pso = mps.tile([128, KSTRIPE], f32, tag="mps",
                                   name=f"pso_{ks}_{qt}")
                    for dp in range(4):
                        group_mms(pso, ks, qt, dp)
                    if j == 3:
                        emit_out_split(pso, ks, qt)
                    else:
                        emit_out(pso, ks, qt, j, late=True, dma_eng=dma_engs[j])

            WAVES = [(0, 0), (0, 4), (1, 0), (1, 4),
                     (0, 8), (1, 8), (0, 12), (1, 12),
                     (2, 0), (2, 4), (2, 8), (2, 12),
                     (3, 0), (3, 4), (3, 8)]
            for wi, (ks, qt_base) in enumerate(WAVES):
                wave(ks, qt_base, late=(wi >= 4), t1_first=(wi == 1))
            wave_seq(3, 12)
    nc.compile()
    return nc


def _stripe_major(x16: np.ndarray) -> np.ndarray:
    # [D, K] -> [KS, 128, DTILES, 512] with [s, p, d, c] = x[d*128+p, s*512+c]
    return np.ascontiguousarray(
        x16.reshape(DTILES, 128, KS, KSTRIPE).transpose(2, 1, 0, 3))


def kernel(q: np.ndarray, k: np.ndarray, embed: np.ndarray) -> np.ndarray:
    import ml_dtypes
    from concourse.bass_utils import run_bass_kernel_spmd

    if "nc" not in _CACHE:
        _CACHE["nc"] = _build()
    nc = _CACHE["nc"]

    bf = ml_dtypes.bfloat16
    f8 = ml_dtypes.float8_e4m3fn
    e = np.asarray(embed[:K], dtype=np.float32)
    eT = e.T
    e8Ts = _stripe_major(eT.astype(f8))
    eTs = _stripe_major(eT.astype(bf))
    in_maps = []
    for b in range(B):
        q8Ts = _stripe_major(np.asarray(q[b], dtype=np.float32).T.astype(f8))
        kpeTs = _stripe_major((np.asarray(k[b], dtype=np.float32) + e).T.astype(bf))
        in_maps.append({"q8Ts": q8Ts, "e8Ts": e8Ts, "eTs": eTs, "kpeTs": kpeTs})
    res = run_bass_kernel_spmd(nc, in_maps, core_ids=list(range(B)), trace=TRACE)
    _CACHE["last_result"] = res
    return np.stack([res.results[b]["out16"].astype(np.float32) for b in range(B)])


# revision 37
# speedup vs baseline: 1.2032x; 1.0060x over previous
"""AbsoluteLearnedPE kernel: data-parallel over batch B, one fused
GEMM-pair per core, PE-roofline-bound, with term-1 in fp8 DoubleRow.

Per core: logits_b = q_b @ E^T + E @ (k_b+E)^T with E = embed[:2048].
Term 1 (q@E^T) runs in fp8e4m3 with perf_mode=DoubleRow (2 contraction
rows per PE cell -> 4 matmuls of K=256 instead of 8 of K=128, ~1.8x
term-1 throughput); term 2 stays bf16. Measured l2 error of the hybrid
is ~1.7e-2 vs the 2e-2 gate (fp8 on BOTH terms is ~2.9e-2 - fails).

Host pre-computes transposes, the k+E add, the bf16/fp8 casts, laid out
stripe-major [KS, 128, DTILES, 512] so a DMA descriptor is [128,
contiguous d-span]. Output is written bf16 and upconverted on host.

Schedule notes (exec window = first useful instruction -> last
instruction end; the ~9.7us end-of-NEFF semaphore-file restore is fixed
overhead):
- DMA queues deliver ~1 descriptor per ~2.2-2.5us nearly independent of
  size, so descriptors are the largest the consumption frontier allows:
  d-pairs for stripe 0, half/whole stripes later (47 input descriptors).
- memset-seeded PE warmups bridge from the prologue to first-chunk
  arrival (a PE idle gap resets the HAM clock ramp; re-warming costs
  ~2.5us at 4/8 rate).
- final wave runs its 4 groups sequentially so 3 of 4 evictions hide
  under matmuls; its output DMAs fan out across queues.
"""

import numpy as np

B, Q, K, D = 8, 2048, 2048, 1024
DTILES = D // 128     # 8
QT = Q // 128         # 16
KSTRIPE = 512
KS = K // KSTRIPE     # 4
WARM_MMS = 15

_CACHE = {}
TRACE = False


def _build():
    from concourse import bacc
    import concourse.mybir as mybir
    import concourse.tile as tile

    f32 = mybir.dt.float32
    bf16 = mybir.dt.bfloat16
    fp8 = mybir.dt.float8e4
    DR = mybir.MatmulPerfMode.DoubleRow

    nc = bacc.Bacc("TRN2", target_bir_lowering=False, debug=False, num_devices=B)
    q8Ts = nc.dram_tensor("q8Ts", [KS, 128, DTILES, KSTRIPE], fp8,
                          kind="ExternalInput").ap()
    e8Ts = nc.dram_tensor("e8Ts", [KS, 128, DTILES, KSTRIPE], fp8,
                          kind="ExternalInput").ap()
    eTs = nc.dram_tensor("eTs", [KS, 128, DTILES, KSTRIPE], bf16,
                         kind="ExternalInput").ap()
    kpeTs = nc.dram_tensor("kpeTs", [KS, 128, DTILES, KSTRIPE], bf16,
                           kind="ExternalInput").ap()
    out16 = nc.dram_tensor("out16", [Q, K], bf16, kind="ExternalOutput").ap()

    with tile.TileContext(nc) as tc:
        with tc.tile_pool(name="big", bufs=1) as big, \
             tc.tile_pool(name="outp", bufs=12) as outp, \
             tc.tile_pool(name="mps", bufs=8, space="PSUM") as mps:

            q8_sb = big.tile([128, KS, DTILES, KSTRIPE], fp8, tag="q8T")
            e8_sb = big.tile([128, KS, DTILES, KSTRIPE], fp8, tag="e8T")
            e_sb = big.tile([128, KS, DTILES, KSTRIPE], bf16, tag="eT")
            kpe_sb = big.tile([128, KS, DTILES, KSTRIPE], bf16, tag="kpeT")

            # PE warmups bridge the DMA lead-in, keeping the HAM clock
            # ramp alive until the first chunks land. (No separate
            # priming descriptors: block A's first pairs absorb the DGE
            # cold-start themselves, and every later descriptor then
            # lands one ~2.3us queue-cadence slot earlier.)
            wtile = big.tile([128, KSTRIPE], bf16, tag="warm")
            nc.gpsimd.memset(wtile[:], 0.0)
            wps = mps.tile([128, KSTRIPE], f32, tag="mps")
            for _ in range(WARM_MMS):
                nc.tensor.matmul(wps[:], wtile[:, 0:128], wtile[:],
                                 start=True, stop=True)

            rr = {"i": 0}
            engs = [nc.sync, nc.scalar, nc.gpsimd]

            def load(sb, dram, s, dlo, dhi):
                eng = engs[rr["i"] % 3]
                rr["i"] += 1
                eng.dma_start(out=sb[:, s, dlo:dhi, :], in_=dram[s, :, dlo:dhi, :])

            # block A: stripe 0 in consumption order. Each group step
            # runs t2 (bf16, needs e+kpe pairs) first and t1 (fp8-DR,
            # needs q8/e8 quads) ~3us later, so the bf16 pairs lead and
            # the half-size fp8 quads slot in behind them. 256KB pair
            # descriptors are the sweet spot: the first descriptor's
            # cold-start latency grows with size (512KB firsts land ~23us
            # and stall the stream), while finer splits lose to the
            # ~2.3us/descriptor queue cadence.
            load(e_sb, eTs, 0, 0, 2)
            load(kpe_sb, kpeTs, 0, 0, 2)
            load(q8_sb, q8Ts, 0, 0, 4)
            load(e8_sb, e8Ts, 0, 0, 4)
            load(e_sb, eTs, 0, 2, 4)
            load(kpe_sb, kpeTs, 0, 2, 4)
            load(e_sb, eTs, 0, 4, 6)
            load(kpe_sb, kpeTs, 0, 4, 6)
            load(q8_sb, q8Ts, 0, 4, 8)
            load(e8_sb, e8Ts, 0, 4, 8)
            load(e_sb, eTs, 0, 6, 8)
            load(kpe_sb, kpeTs, 0, 6, 8)
            # block B: wave (0,qt4-7) lhsT needs q8 s1 (t1) + e s1 (t2).
            # q8 s1 is split in two halves on different queues so the
            # wave's first t1 d-pairs gate on the earlier half only —
            # halves the jitter exposure against its ~25us deadline.
            load(q8_sb, q8Ts, 1, 0, 4)
            load(q8_sb, q8Ts, 1, 4, 8)
            load(e_sb, eTs, 1, 0, 4)
            load(e_sb, eTs, 1, 4, 8)
            # blocks C-E: whole-stripe DMAs, need-order.
            load(e8_sb, e8Ts, 1, 0, 8)       # wave (1,0) t1 rhs
            load(kpe_sb, kpeTs, 1, 0, 8)     # wave (1,0) t2 rhs
            load(q8_sb, q8Ts, 2, 0, 8)       # wave (0,8) lhsT
            load(e_sb, eTs, 2, 0, 8)
            load(q8_sb, q8Ts, 3, 0, 8)       # wave (0,12) lhsT
            load(e_sb, eTs, 3, 0, 8)
            load(e8_sb, e8Ts, 2, 0, 8)       # waves (2,*) rhs
            load(kpe_sb, kpeTs, 2, 0, 8)
            load(e8_sb, e8Ts, 3, 0, 8)       # waves (3,*) rhs
            load(kpe_sb, kpeTs, 3, 0, 8)

            def emit_out(pso, ks, qt, j=0, late=False, dma_eng=None):
                o_t = outp.tile([128, KSTRIPE], bf16, tag="o_t", name="o_t")
                if late and j % 2 == 1:
                    nc.scalar.copy(out=o_t[:], in_=pso[:])
                else:
                    nc.vector.tensor_copy(out=o_t[:], in_=pso[:])
                (dma_eng or nc.sync).dma_start(
                    out=out16[qt * 128:(qt + 1) * 128,
                              ks * KSTRIPE:(ks + 1) * KSTRIPE],
                    in_=o_t[:])

            def t2_mm(pso, ks, qt, d, start=False, stop=False):
                sq, cq = qt // 4, qt % 4
                qs = slice(cq * 128, (cq + 1) * 128)
                nc.tensor.matmul(pso[:], e_sb[:, sq, d, qs],
                                 kpe_sb[:, ks, d, :], start=start, stop=stop)

            def t1_mm(pso, ks, qt, dp, start=False, stop=False):
                sq, cq = qt // 4, qt % 4
                qs = slice(cq * 128, (cq + 1) * 128)
                d = 2 * dp
                nc.tensor.matmul(pso[:], q8_sb[:, sq, d:d + 2, qs],
                                 e8_sb[:, ks, d:d + 2, :],
                                 start=start, stop=stop, perf_mode=DR)

            def group_mms(pso, ks, qt, dp):
                # One d-pair step of one group: 2 bf16 matmuls (t2) then
                # 1 fp8-DR matmul (t1, contraction 256). t2 leads because
                # its chunks arrive first during the ramp.
                d = 2 * dp
                t2_mm(pso, ks, qt, d, start=(dp == 0))
                t2_mm(pso, ks, qt, d + 1)
                t1_mm(pso, ks, qt, dp, stop=(dp == 3))

            def wave(ks, qt_base, late=False, t1_first=False):
                # 4 groups, d-pair-major interleaved across groups so each
                # delivered chunk set unlocks 12 matmuls: the 8 t2 mms of
                # a step run before its 4 t1 mms. t1_first flips that —
                # used for wave (0,4), whose t1 needs only the small q8 s1
                # block (rhs e8 s0 is resident) while its t2 lhsT (e s1,
                # 1MB halves) lands ~3us later.
                qts = [qt_base + j for j in range(4)]
                psos = [mps.tile([128, KSTRIPE], f32, tag="mps",
                                 name=f"pso_{ks}_{qt}") for qt in qts]
                if t1_first:
                    for dp in range(4):
                        for j, qt in enumerate(qts):
                            t1_mm(psos[j], ks, qt, dp, start=(dp == 0))
                    for d in range(DTILES):
                        for j, qt in enumerate(qts):
                            t2_mm(psos[j], ks, qt, d, stop=(d == DTILES - 1))
                else:
                    for dp in range(4):
                        d = 2 * dp
                        for j, qt in enumerate(qts):
                            t2_mm(psos[j], ks, qt, d, start=(dp == 0))
                            t2_mm(psos[j], ks, qt, d + 1)
                        for j, qt in enumerate(qts):
                            t1_mm(psos[j], ks, qt, dp, stop=(dp == 3))
                for j, qt in enumerate(qts):
                    emit_out(psos[j], ks, qt, j, late)

            def emit_out_split(pso, ks, qt):
                # Minimal-latency eviction for the very last group: two
                # half-tiles cast on Vector+Scalar in parallel, two
                # half-DMAs on Sync+GpSimd in parallel — halves the
                # post-stream drain vs the single-tile path.
                h = KSTRIPE // 2
                o_t = outp.tile([128, KSTRIPE], bf16, tag="o_t", name="o_t")
                nc.vector.tensor_copy(out=o_t[:, 0:h], in_=pso[:, 0:h])
                nc.scalar.copy(out=o_t[:, h:], in_=pso[:, h:])
                rows = slice(qt * 128, (qt + 1) * 128)
                # Both halves ride the Sync HW ring back-to-back: the
                # GpSimd SW ring issues ~0.4us late and was the drain's
                # critical path.
                nc.sync.dma_start(
                    out=out16[rows, ks * KSTRIPE:ks * KSTRIPE + h],
                    in_=o_t[:, 0:h])
                nc.sync.dma_start(
                    out=out16[rows, ks * KSTRIPE + h:(ks + 1) * KSTRIPE],
                    in_=o_t[:, h:])

            def wave_seq(ks, qt_base):
                # Final wave: groups sequential so evictions overlap the
                # stream; output DMAs fan out across queues.
                dma_engs = [nc.sync, nc.gpsimd, nc.gpsimd, nc.sync]
                for j, qt in enumerate(qt_base + jj for jj in range(4)):
                    pso = mps.tile([128, KSTRIPE], f32, tag="mps",
                                   name=f"pso_{ks}_{qt}")
                    for dp in range(4):
                        group_mms(pso, ks, qt, dp)
                    if j == 3:
                        emit_out_split(pso, ks, qt)
                    else:
                        emit_out(pso, ks, qt, j, late=True, dma_eng=dma_engs[j])

            WAVES = [(0, 0), (0, 4), (1, 0), (1, 4),
                     (0, 8), (1, 8), (0, 12), (1, 12),
                     (2, 0), (2, 4), (2, 8), (2, 12),
                     (3, 0), (3, 4), (3, 8)]
            for wi, (ks, qt_base) in enumerate(WAVES):
                wave(ks, qt_base, late=(wi >= 4), t1_first=(wi == 1))
            wave_seq(3, 12)
    nc.compile()
    return nc


def _stripe_major(x16: np.ndarray) -> np.ndarray:
    # [D, K] -> [KS, 128, DTILES, 512] with [s, p, d, c] = x[d*128+p, s*512+c]
    return np.ascontiguousarray(
        x16.reshape(DTILES, 128, KS, KSTRIPE).transpose(2, 1, 0, 3))


def kernel(q: np.ndarray, k: np.ndarray, embed: np.ndarray) -> np.ndarray:
    import ml_dtypes
    from concourse.bass_utils import run_bass_kernel_spmd

    if "nc" not in _CACHE:
        _CACHE["nc"] = _build()
    nc = _CACHE["nc"]

    bf = ml_dtypes.bfloat16
    f8 = ml_dtypes.float8_e4m3fn
    e = np.asarray(embed[:K], dtype=np.float32)
    eT = e.T
    e8Ts = _stripe_major(eT.astype(f8))
    eTs = _stripe_major(eT.astype(bf))
    in_maps = []
    for b in range(B):
        q8Ts = _stripe_major(np.asarray(q[b], dtype=np.float32).T.astype(f8))
        kpeTs = _stripe_major((np.asarray(k[b], dtype=np.float32) + e).T.astype(bf))
        in_maps.append({"q8Ts": q8Ts, "e8Ts": e8Ts, "eTs": eTs, "kpeTs": kpeTs})
    res = run_bass_kernel_spmd(nc, in_maps, core_ids=list(range(B)), trace=TRACE)
    _CACHE["last_result"] = res
    return np.stack([res.results[b]["out16"].astype(np.float32) for b in range(B)])
